# revision 9
# baseline (speedup 1.0000x reference)
"""Trainium2 Bass kernel for nn_Correlation_network.

Math (B=1, H=W=80, C=256, R=H*W=6400):
  M[r1, r2]   = sum_c A[r1, c] * B[r2, c]          (r = (i*80+j) flattened)
  corr        = M / C
  penalty[r]  = sqrt(sum_{r2} corr[r, r2]^2) = sqrt(q[r]) / C,  q[r] = sum_{r2} M[r, r2]^2
  out[r1, r2] = corr[r1, r2] / penalty[r2]         (TF trailing-dim broadcast -> column index!)
              = M[r1, r2] / sqrt(q[r2])            (C cancels exactly)

Key trick: q[r] = a_r^T G a_r with G = B_flat^T B_flat (256x256), so every
device computes the FULL q vector locally from small matmuls -- no
cross-device communication, no full 6400x6400 matmul per device.

Sharding: rows r1 split across 8 cores (800 each, = H/8 query rows).
Each device computes out[shard, :] = M[shard, :] * rsqrt(q)[None, :].
"""

import sys

if "/opt/trn_rl_repo" not in sys.path:
    sys.path.insert(0, "/opt/trn_rl_repo")

import numpy as np

H = 80
W = 80
C = 256
R = H * W          # 6400
N_CORES = 8
S = R // N_CORES   # 800 output rows per core

NW = 512           # r2 chunk width (PSUM bank = 512 fp32)
N_CHUNKS = [(n0, min(NW, R - n0)) for n0 in range(0, R, NW)]          # 12x512 + 256
M_TILES = [(m0, min(128, S - m0)) for m0 in range(0, S, 128)]         # 6x128 + 32
KB = C // 128      # 2 contraction sub-tiles


def _np_dt(name):
    if name == "bfloat16":
        import ml_dtypes

        return ml_dtypes.bfloat16
    return np.float32


def build_bass(mm_dt_name: str = "float32r", pen_dt_name: str = "bfloat16"):
    """Build the SPMD Bass program (one program, run on all 8 cores).

    mm_dt_name:  dtype of the main M = A_shard @ B^T matmul inputs.
    pen_dt_name: dtype of the penalty path (G, Tt) matmul inputs.
    float32r tensors are stored/DMA'd as fp32 bits but typed float32r end to
    end so the BIR verifier sees fp32r-typed producers.
    The rsqrt + broadcast of the scale vector stays exact fp32.
    """
    import concourse.tile as tile
    from concourse import bacc, mybir

    f32 = mybir.dt.float32
    bf16 = mybir.dt.bfloat16
    mm_dt = getattr(mybir.dt, mm_dt_name)
    pen_dt = getattr(mybir.dt, pen_dt_name)
    # dtype of the elementwise-product operand feeding the q reduction matmul
    q_dt = f32 if pen_dt == f32 else bf16

    def dve(ap):
        # view for vector-engine consumption (fp32r is just fp32 bits)
        return ap.bitcast(f32) if ap.dtype == mybir.dt.float32r else ap

    nc = bacc.Bacc(None, target_bir_lowering=False)

    at = nc.declare_dram_parameter("at", [C, R], pen_dt, isOutput=False)     # A^T full
    bt = nc.declare_dram_parameter("bt", [C, R], mm_dt, isOutput=False)      # B^T full
    b = nc.declare_dram_parameter("b", [R, C], pen_dt, isOutput=False)       # B row-major
    ash = nc.declare_dram_parameter("ashard", [C, S], mm_dt, isOutput=False)  # A^T[:, shard]
    out = nc.declare_dram_parameter("out", [S, R], f32, isOutput=True)

    with tile.TileContext(nc) as tc:
        with (
            tc.tile_pool(name="big", bufs=1) as big,
            tc.tile_pool(name="consts", bufs=1) as consts,
            tc.tile_pool(name="pwork", bufs=3) as pwork,
            tc.tile_pool(name="outp", bufs=5) as outp,
            tc.tile_pool(name="ps_acc", bufs=2, space="PSUM") as ps_acc,   # G / Tt
            tc.tile_pool(name="ps_q", bufs=2, space="PSUM") as ps_q,       # q rows
            tc.tile_pool(name="ps_main", bufs=4, space="PSUM") as ps_main, # bcast + main
        ):
            # ---- input loads -------------------------------------------------
            at_sb = big.tile([128, KB, R], pen_dt)
            nc.sync.dma_start(out=at_sb, in_=at[:, :].rearrange("(k p) r -> p k r", p=128))
            bt_sb = big.tile([128, KB, R], mm_dt)
            nc.sync.dma_start(out=bt_sb, in_=bt[:, :].rearrange("(k p) r -> p k r", p=128))
            ash_sb = big.tile([128, KB, S], mm_dt)
            nc.sync.dma_start(out=ash_sb, in_=ash[:, :].rearrange("(k p) m -> p k m", p=128))

            ones_col = consts.tile([128, 1], q_dt)
            nc.vector.memset(ones_col, 1.0)
            ones_row = consts.tile([1, 128], f32)
            nc.vector.memset(ones_row, 1.0)

            g_sb = consts.tile([128, KB, C], pen_dt)    # G[c, c'] (c = k*128+p)
            scale_bc = consts.tile([128, R], f32)       # rsqrt(q) broadcast to 128 partitions

            # ---- phase 1: G = B^T B  (accumulate over 50 row-chunks of B) ----
            n_rchunks = R // 128  # 50
            b_sb = big.tile([128, n_rchunks, C], pen_dt)
            nc.sync.dma_start(out=b_sb, in_=b[:, :].rearrange("(t p) c -> p t c", p=128))
            g_ps = [
                ps_acc.tile([128, NW], f32, tag="acc", name=f"g_ps{kk}")[:, :C]
                for kk in range(KB)
            ]
            for t in range(n_rchunks):
                b_t = b_sb[:, t, :]
                for kk in range(KB):
                    nc.tensor.matmul(
                        g_ps[kk],
                        lhsT=b_t[:, kk * 128:(kk + 1) * 128],
                        rhs=b_t,
                        start=(t == 0),
                        stop=(t == n_rchunks - 1),
                    )
            for kk in range(KB):
                nc.vector.tensor_copy(g_sb[:, kk, :], g_ps[kk])

            # ---- phase 2: q[r] = a_r^T G a_r, scale = rsqrt(q), bcast -------
            for (n0, nw) in N_CHUNKS:
                q_ps = ps_q.tile([1, NW], f32, tag="q")
                for kk in range(KB):        # output c-half of Tt = G @ A^T
                    tt_ps = ps_acc.tile([128, NW], f32, tag="acc")
                    for k in range(KB):     # contraction over c'
                        nc.tensor.matmul(
                            tt_ps[:, :nw],
                            lhsT=g_sb[:, k, kk * 128:(kk + 1) * 128],
                            rhs=at_sb[:, k, n0:n0 + nw],
                            start=(k == 0),
                            stop=(k == KB - 1),
                        )
                    p_sb = pwork.tile([128, NW], q_dt, tag="p")
                    nc.vector.tensor_mul(p_sb[:, :nw], tt_ps[:, :nw], dve(at_sb[:, kk, n0:n0 + nw]))
                    nc.tensor.matmul(
                        q_ps[:, :nw],
                        lhsT=ones_col,
                        rhs=p_sb[:, :nw],
                        start=(kk == 0),
                        stop=(kk == KB - 1),
                    )
                srow = pwork.tile([1, NW], f32, tag="srow")
                nc.scalar.activation(
                    out=srow[:, :nw], in_=q_ps[:, :nw],
                    func=mybir.ActivationFunctionType.Sqrt,
                )
                nc.vector.reciprocal(srow[:, :nw], srow[:, :nw])
                bc_ps = ps_main.tile([128, NW], f32, tag="main")
                nc.tensor.matmul(
                    bc_ps[:, :nw],
                    lhsT=ones_row,
                    rhs=srow[:, :nw],
                    start=True,
                    stop=True,
                )
                nc.vector.tensor_copy(scale_bc[:, n0:n0 + nw], bc_ps[:, :nw])

            # ---- phase 3: out[shard] = (A_shard @ B^T) * scale[None, :] -----
            for (m0, mp) in M_TILES:
                for (n0, nw) in N_CHUNKS:
                    o_ps = ps_main.tile([128, NW], f32, tag="main")
                    for k in range(KB):
                        nc.tensor.matmul(
                            o_ps[:mp, :nw],
                            lhsT=ash_sb[:, k, m0:m0 + mp],
                            rhs=bt_sb[:, k, n0:n0 + nw],
                            start=(k == 0),
                            stop=(k == KB - 1),
                        )
                    o_sb = outp.tile([128, NW], f32, tag="o")
                    nc.any.tensor_mul(o_sb[:mp, :nw], o_ps[:mp, :nw], scale_bc[:mp, n0:n0 + nw])
                    nc.sync.dma_start(out=out[m0:m0 + mp, n0:n0 + nw], in_=o_sb[:mp, :nw])

    if not nc.is_finalized():
        nc.finalize()
    return nc


_NC_CACHE = {}


def _get_nc(mm_dt_name="float32r", pen_dt_name="bfloat16"):
    key = (mm_dt_name, pen_dt_name)
    if key not in _NC_CACHE:
        _NC_CACHE[key] = build_bass(*key)
    return _NC_CACHE[key]


def _install_ntff_hook_shim():
    """The agent image lacks antenv.axon_hooks; recreate the glue so
    trace=True can capture NTFF profiles via libaxon_pjrt.so ctypes."""
    import types

    name = "antenv.axon_hooks"
    if name in sys.modules:
        return
    hook = None
    try:
        from trn_agent_boot.trn_boot import _ntff_profile_via_ctypes

        hook = _ntff_profile_via_ctypes("/opt/axon/libaxon_pjrt.so")
    except Exception as e:  # pragma: no cover - profiling is best-effort
        print("ntff hook shim unavailable:", e)
    mod = types.ModuleType(name)
    mod._hook = hook
    mod.get_axon_ntff_profile_hook = lambda: mod._hook
    mod.set_axon_ntff_profile_hook = lambda h: setattr(mod, "_hook", h)
    sys.modules[name] = mod
    import antenv

    antenv.axon_hooks = mod


def _run(feature_A, feature_B, trace=False, mm_dt_name="float32r", pen_dt_name="bfloat16"):
    from concourse.bass_utils import run_bass_kernel_spmd

    if trace:
        _install_ntff_hook_shim()

    mm_np = _np_dt(mm_dt_name)
    pen_np = _np_dt(pen_dt_name)

    A = np.ascontiguousarray(np.asarray(feature_A, dtype=np.float32).reshape(R, C))
    B = np.ascontiguousarray(np.asarray(feature_B, dtype=np.float32).reshape(R, C))
    AT = np.ascontiguousarray(A.T)
    BT = np.ascontiguousarray(B.T).astype(mm_np)

    at_in = AT.astype(pen_np)
    b_in = B.astype(pen_np)

    in_maps = []
    for d in range(N_CORES):
        in_maps.append({
            "at": at_in,
            "bt": BT,
            "b": b_in,
            "ashard": np.ascontiguousarray(AT[:, d * S:(d + 1) * S]).astype(mm_np),
        })

    nc = _get_nc(mm_dt_name, pen_dt_name)
    res = run_bass_kernel_spmd(nc, in_maps, core_ids=list(range(N_CORES)), trace=trace)
    full = np.concatenate([r["out"] for r in res.results], axis=0)  # [R, R]
    return full.reshape(1, H, W, H, W), res


def kernel(feature_A, feature_B):
    out, _ = _run(feature_A, feature_B, trace=False)
    return out


# revision 13
# speedup vs baseline: 1.3144x; 1.3144x over previous
"""Trainium2 Bass kernel for nn_Correlation_network.

Math (B=1, H=W=80, C=256, R=H*W=6400):
  M[r1, r2]   = sum_c A[r1, c] * B[r2, c]          (r = (i*80+j) flattened)
  corr        = M / C
  penalty[r]  = sqrt(sum_{r2} corr[r, r2]^2) = sqrt(q[r]) / C,  q[r] = sum_{r2} M[r, r2]^2
  out[r1, r2] = corr[r1, r2] / penalty[r2]         (TF trailing-dim broadcast -> column index!)
              = M[r1, r2] / sqrt(q[r2])            (C cancels exactly)

Key trick: q[r] = a_r^T G a_r with G = B_flat^T B_flat (256x256), so every
device computes the FULL q vector locally from small matmuls -- no
cross-device communication, no full 6400x6400 matmul per device.

Sharding: rows r1 split across 8 cores (800 each, = H/8 query rows).
Each device computes out[shard, :] = M[shard, :] * rsqrt(q)[None, :].
"""

import sys

if "/opt/trn_rl_repo" not in sys.path:
    sys.path.insert(0, "/opt/trn_rl_repo")

import numpy as np

H = 80
W = 80
C = 256
R = H * W          # 6400
N_CORES = 8
S = R // N_CORES   # 800 output rows per core

NW = 512           # r2 chunk width (PSUM bank = 512 fp32)
N_CHUNKS = [(n0, min(NW, R - n0)) for n0 in range(0, R, NW)]          # 12x512 + 256
M_TILES = [(m0, min(128, S - m0)) for m0 in range(0, S, 128)]         # 6x128 + 32
KB = C // 128      # 2 contraction sub-tiles


def _np_dt(name):
    if name == "bfloat16":
        import ml_dtypes

        return ml_dtypes.bfloat16
    return np.float32


def build_bass(mm_dt_name: str = "float32r", pen_dt_name: str = "bfloat16"):
    """Build the SPMD Bass program (one program, run on all 8 cores).

    mm_dt_name:  dtype of the main M = A_shard @ B^T matmul inputs.
    pen_dt_name: dtype of the penalty path (G, Tt) matmul inputs.
    float32r tensors are stored/DMA'd as fp32 bits but typed float32r end to
    end so the BIR verifier sees fp32r-typed producers.
    The rsqrt + broadcast of the scale vector stays exact fp32.
    """
    import concourse.tile as tile
    from concourse import bacc, mybir

    f32 = mybir.dt.float32
    bf16 = mybir.dt.bfloat16
    mm_dt = getattr(mybir.dt, mm_dt_name)
    pen_dt = getattr(mybir.dt, pen_dt_name)
    # dtype of the elementwise-product operand feeding the q reduction matmul
    q_dt = f32 if pen_dt == f32 else bf16

    def dve(ap):
        # view for vector-engine consumption (fp32r is just fp32 bits)
        return ap.bitcast(f32) if ap.dtype == mybir.dt.float32r else ap

    nc = bacc.Bacc(None, target_bir_lowering=False)

    at = nc.declare_dram_parameter("at", [C, R], pen_dt, isOutput=False)     # A^T full
    bt = nc.declare_dram_parameter("bt", [C, R], mm_dt, isOutput=False)      # B^T full
    b = nc.declare_dram_parameter("b", [R, C], pen_dt, isOutput=False)       # B row-major
    ash = nc.declare_dram_parameter("ashard", [C, S], mm_dt, isOutput=False)  # A^T[:, shard]
    out = nc.declare_dram_parameter("out", [S, R], f32, isOutput=True)

    with tile.TileContext(nc) as tc:
        with (
            tc.tile_pool(name="big", bufs=1) as big,
            tc.tile_pool(name="consts", bufs=1) as consts,
            tc.tile_pool(name="pwork", bufs=3) as pwork,
            tc.tile_pool(name="outp", bufs=5) as outp,
            tc.tile_pool(name="ps_acc", bufs=2, space="PSUM") as ps_acc,   # G / Tt
            tc.tile_pool(name="ps_q", bufs=2, space="PSUM") as ps_q,       # q rows
            tc.tile_pool(name="ps_main", bufs=4, space="PSUM") as ps_main, # bcast + main
        ):
            # ---- input loads (order matters: G needs b first; bt is needed
            # last and per-chunk, so it streams while G/penalty compute) -----
            n_rchunks = R // 128  # 50
            b_sb = big.tile([128, n_rchunks, C], pen_dt)
            nc.sync.dma_start(out=b_sb, in_=b[:, :].rearrange("(t p) c -> p t c", p=128))
            at_sb = big.tile([128, KB, R], pen_dt)
            at_src = at[:, :].rearrange("(k p) r -> p k r", p=128)
            for a0 in range(0, R, 1600):
                nc.sync.dma_start(out=at_sb[:, :, a0:a0 + 1600], in_=at_src[:, :, a0:a0 + 1600])
            ash_sb = big.tile([128, KB, S], mm_dt)
            nc.sync.dma_start(out=ash_sb, in_=ash[:, :].rearrange("(k p) m -> p k m", p=128))
            bt_sb = big.tile([128, KB, R], mm_dt)
            bt_src = bt[:, :].rearrange("(k p) r -> p k r", p=128)
            for (n0, nw) in N_CHUNKS:
                nc.sync.dma_start(out=bt_sb[:, :, n0:n0 + nw], in_=bt_src[:, :, n0:n0 + nw])

            ones_col = consts.tile([128, 1], q_dt)
            nc.vector.memset(ones_col, 1.0)
            ones_row = consts.tile([1, 128], f32)
            nc.vector.memset(ones_row, 1.0)

            g_sb = consts.tile([128, KB, C], pen_dt)    # G[c, c'] (c = k*128+p)
            scale_bc = consts.tile([128, R], f32)       # rsqrt(q) broadcast to 128 partitions

            # ---- phase 1: G = B^T B  (accumulate over 50 row-chunks of B) ----
            g_ps = [
                ps_acc.tile([128, NW], f32, tag="acc", name=f"g_ps{kk}")[:, :C]
                for kk in range(KB)
            ]
            for t in range(n_rchunks):
                b_t = b_sb[:, t, :]
                for kk in range(KB):
                    nc.tensor.matmul(
                        g_ps[kk],
                        lhsT=b_t[:, kk * 128:(kk + 1) * 128],
                        rhs=b_t,
                        start=(t == 0),
                        stop=(t == n_rchunks - 1),
                    )
            for kk in range(KB):
                nc.vector.tensor_copy(g_sb[:, kk, :], g_ps[kk])

            # ---- phase 2: q[r] = a_r^T G a_r, scale = rsqrt(q), bcast -------
            for (n0, nw) in N_CHUNKS:
                q_ps = ps_q.tile([1, NW], f32, tag="q")
                for kk in range(KB):        # output c-half of Tt = G @ A^T
                    tt_ps = ps_acc.tile([128, NW], f32, tag="acc")
                    for k in range(KB):     # contraction over c'
                        nc.tensor.matmul(
                            tt_ps[:, :nw],
                            lhsT=g_sb[:, k, kk * 128:(kk + 1) * 128],
                            rhs=at_sb[:, k, n0:n0 + nw],
                            start=(k == 0),
                            stop=(k == KB - 1),
                        )
                    p_sb = pwork.tile([128, NW], q_dt, tag="p")
                    nc.vector.tensor_mul(p_sb[:, :nw], tt_ps[:, :nw], dve(at_sb[:, kk, n0:n0 + nw]))
                    nc.tensor.matmul(
                        q_ps[:, :nw],
                        lhsT=ones_col,
                        rhs=p_sb[:, :nw],
                        start=(kk == 0),
                        stop=(kk == KB - 1),
                    )
                srow = pwork.tile([1, NW], f32, tag="srow")
                nc.scalar.activation(
                    out=srow[:, :nw], in_=q_ps[:, :nw],
                    func=mybir.ActivationFunctionType.Sqrt,
                )
                # ~51-ULP reciprocal, 5x faster than exact (error << matmul dtype error)
                srow2 = pwork.tile([1, NW], f32, tag="srow2")
                nc.vector.reciprocal_approx_fast(out=srow2[:, :nw], in_=srow[:, :nw])
                srow = srow2
                bc_ps = ps_main.tile([128, NW], f32, tag="main")
                nc.tensor.matmul(
                    bc_ps[:, :nw],
                    lhsT=ones_row,
                    rhs=srow[:, :nw],
                    start=True,
                    stop=True,
                )
                nc.vector.tensor_copy(scale_bc[:, n0:n0 + nw], bc_ps[:, :nw])

            # ---- phase 3: out[shard] = (A_shard @ B^T) * scale[None, :] -----
            # n-outer: chunk n's work starts as soon as its bt chunk + scale
            # chunk are ready, overlapping with the penalty phase.
            for (n0, nw) in N_CHUNKS:
                for (m0, mp) in M_TILES:
                    o_ps = ps_main.tile([128, NW], f32, tag="main")
                    for k in range(KB):
                        nc.tensor.matmul(
                            o_ps[:mp, :nw],
                            lhsT=ash_sb[:, k, m0:m0 + mp],
                            rhs=bt_sb[:, k, n0:n0 + nw],
                            start=(k == 0),
                            stop=(k == KB - 1),
                        )
                    o_sb = outp.tile([128, NW], f32, tag="o")
                    nc.any.tensor_mul(o_sb[:mp, :nw], o_ps[:mp, :nw], scale_bc[:mp, n0:n0 + nw])
                    nc.sync.dma_start(out=out[m0:m0 + mp, n0:n0 + nw], in_=o_sb[:mp, :nw])

    if not nc.is_finalized():
        nc.finalize()
    return nc


_NC_CACHE = {}


def _get_nc(mm_dt_name="float32r", pen_dt_name="bfloat16"):
    key = (mm_dt_name, pen_dt_name)
    if key not in _NC_CACHE:
        _NC_CACHE[key] = build_bass(*key)
    return _NC_CACHE[key]


def _install_ntff_hook_shim():
    """The agent image lacks antenv.axon_hooks; recreate the glue so
    trace=True can capture NTFF profiles via libaxon_pjrt.so ctypes."""
    import types

    name = "antenv.axon_hooks"
    if name in sys.modules:
        return
    hook = None
    try:
        from trn_agent_boot.trn_boot import _ntff_profile_via_ctypes

        hook = _ntff_profile_via_ctypes("/opt/axon/libaxon_pjrt.so")
    except Exception as e:  # pragma: no cover - profiling is best-effort
        print("ntff hook shim unavailable:", e)
    mod = types.ModuleType(name)
    mod._hook = hook
    mod.get_axon_ntff_profile_hook = lambda: mod._hook
    mod.set_axon_ntff_profile_hook = lambda h: setattr(mod, "_hook", h)
    sys.modules[name] = mod
    import antenv

    antenv.axon_hooks = mod


def _run(feature_A, feature_B, trace=False, mm_dt_name="float32r", pen_dt_name="bfloat16"):
    from concourse.bass_utils import run_bass_kernel_spmd

    if trace:
        _install_ntff_hook_shim()

    mm_np = _np_dt(mm_dt_name)
    pen_np = _np_dt(pen_dt_name)

    A = np.ascontiguousarray(np.asarray(feature_A, dtype=np.float32).reshape(R, C))
    B = np.ascontiguousarray(np.asarray(feature_B, dtype=np.float32).reshape(R, C))
    AT = np.ascontiguousarray(A.T)
    BT = np.ascontiguousarray(B.T).astype(mm_np)

    at_in = AT.astype(pen_np)
    b_in = B.astype(pen_np)

    in_maps = []
    for d in range(N_CORES):
        in_maps.append({
            "at": at_in,
            "bt": BT,
            "b": b_in,
            "ashard": np.ascontiguousarray(AT[:, d * S:(d + 1) * S]).astype(mm_np),
        })

    nc = _get_nc(mm_dt_name, pen_dt_name)
    res = run_bass_kernel_spmd(nc, in_maps, core_ids=list(range(N_CORES)), trace=trace)
    full = np.concatenate([r["out"] for r in res.results], axis=0)  # [R, R]
    return full.reshape(1, H, W, H, W), res


def kernel(feature_A, feature_B):
    out, _ = _run(feature_A, feature_B, trace=False)
    return out


# revision 15
# speedup vs baseline: 1.3195x; 1.0039x over previous
"""Trainium2 Bass kernel for nn_Correlation_network.

Math (B=1, H=W=80, C=256, R=H*W=6400):
  M[r1, r2]   = sum_c A[r1, c] * B[r2, c]          (r = (i*80+j) flattened)
  corr        = M / C
  penalty[r]  = sqrt(sum_{r2} corr[r, r2]^2) = sqrt(q[r]) / C,  q[r] = sum_{r2} M[r, r2]^2
  out[r1, r2] = corr[r1, r2] / penalty[r2]         (TF trailing-dim broadcast -> column index!)
              = M[r1, r2] / sqrt(q[r2])            (C cancels exactly)

Key trick: q[r] = a_r^T G a_r with G = B_flat^T B_flat (256x256), so every
device computes the FULL q vector locally from small matmuls -- no
cross-device communication, no full 6400x6400 matmul per device.

Sharding: rows r1 split across 8 cores (800 each, = H/8 query rows).
Each device computes out[shard, :] = M[shard, :] * rsqrt(q)[None, :].
"""

import sys

if "/opt/trn_rl_repo" not in sys.path:
    sys.path.insert(0, "/opt/trn_rl_repo")

import numpy as np

H = 80
W = 80
C = 256
R = H * W          # 6400
N_CORES = 8
S = R // N_CORES   # 800 output rows per core

NW = 512           # r2 chunk width (PSUM bank = 512 fp32)
N_CHUNKS = [(n0, min(NW, R - n0)) for n0 in range(0, R, NW)]          # 12x512 + 256
M_TILES = [(m0, min(128, S - m0)) for m0 in range(0, S, 128)]         # 6x128 + 32
KB = C // 128      # 2 contraction sub-tiles


def _np_dt(name):
    if name == "bfloat16":
        import ml_dtypes

        return ml_dtypes.bfloat16
    return np.float32


def build_bass(mm_dt_name: str = "float32r", pen_dt_name: str = "bfloat16"):
    """Build the SPMD Bass program (one program, run on all 8 cores).

    mm_dt_name:  dtype of the main M = A_shard @ B^T matmul inputs.
    pen_dt_name: dtype of the penalty path (G, Tt) matmul inputs.
    float32r tensors are stored/DMA'd as fp32 bits but typed float32r end to
    end so the BIR verifier sees fp32r-typed producers.
    The rsqrt + broadcast of the scale vector stays exact fp32.
    """
    import concourse.tile as tile
    from concourse import bacc, mybir

    f32 = mybir.dt.float32
    bf16 = mybir.dt.bfloat16
    mm_dt = getattr(mybir.dt, mm_dt_name)
    pen_dt = getattr(mybir.dt, pen_dt_name)
    # dtype of the elementwise-product operand feeding the q reduction matmul
    q_dt = f32 if pen_dt == f32 else bf16

    def dve(ap):
        # view for vector-engine consumption (fp32r is just fp32 bits)
        return ap.bitcast(f32) if ap.dtype == mybir.dt.float32r else ap

    nc = bacc.Bacc(None, target_bir_lowering=False)

    at = nc.declare_dram_parameter("at", [C, R], pen_dt, isOutput=False)     # A^T full
    bt = nc.declare_dram_parameter("bt", [C, R], mm_dt, isOutput=False)      # B^T full
    b = nc.declare_dram_parameter("b", [R, C], pen_dt, isOutput=False)       # B row-major
    ash = nc.declare_dram_parameter("ashard", [C, S], mm_dt, isOutput=False)  # A^T[:, shard]
    out = nc.declare_dram_parameter("out", [S, R], f32, isOutput=True)

    with tile.TileContext(nc) as tc:
        with (
            tc.tile_pool(name="big", bufs=1) as big,
            tc.tile_pool(name="consts", bufs=1) as consts,
            tc.tile_pool(name="pwork", bufs=3) as pwork,
            tc.tile_pool(name="outp", bufs=5) as outp,
            tc.tile_pool(name="ps_acc", bufs=2, space="PSUM") as ps_acc,   # G / Tt
            tc.tile_pool(name="ps_q", bufs=1, space="PSUM") as ps_q,       # q rows
            tc.tile_pool(name="ps_main", bufs=5, space="PSUM") as ps_main, # bcast + main
        ):
            # ---- input loads (order matters: G needs b first; bt is needed
            # last and per-chunk, so it streams while G/penalty compute) -----
            n_rchunks = R // 128  # 50
            b_sb = big.tile([128, n_rchunks, C], pen_dt)
            b_src = b[:, :].rearrange("(t p) c -> p t c", p=128)
            for t0 in range(0, n_rchunks, 13):
                t1 = min(t0 + 13, n_rchunks)
                nc.sync.dma_start(out=b_sb[:, t0:t1, :], in_=b_src[:, t0:t1, :])
            at_sb = big.tile([128, KB, R], pen_dt)
            at_src = at[:, :].rearrange("(k p) r -> p k r", p=128)
            for a0 in range(0, R, 1600):
                nc.sync.dma_start(out=at_sb[:, :, a0:a0 + 1600], in_=at_src[:, :, a0:a0 + 1600])
            ash_sb = big.tile([128, KB, S], mm_dt)
            nc.sync.dma_start(out=ash_sb, in_=ash[:, :].rearrange("(k p) m -> p k m", p=128))
            bt_sb = big.tile([128, KB, R], mm_dt)
            bt_src = bt[:, :].rearrange("(k p) r -> p k r", p=128)
            for (n0, nw) in N_CHUNKS:
                nc.sync.dma_start(out=bt_sb[:, :, n0:n0 + nw], in_=bt_src[:, :, n0:n0 + nw])

            ones_col = consts.tile([128, 1], q_dt)
            nc.vector.memset(ones_col, 1.0)
            ones_row = consts.tile([1, 128], f32)
            nc.vector.memset(ones_row, 1.0)

            g_sb = consts.tile([128, KB, C], pen_dt)    # G[c, c'] (c = k*128+p)
            scale_bc = consts.tile([128, R], f32)       # rsqrt(q) broadcast to 128 partitions

            # ---- phase 1: G = B^T B  (accumulate over 50 row-chunks of B) ----
            g_ps = [
                ps_acc.tile([128, NW], f32, tag="acc", name=f"g_ps{kk}")[:, :C]
                for kk in range(KB)
            ]
            for t in range(n_rchunks):
                b_t = b_sb[:, t, :]
                for kk in range(KB):
                    nc.tensor.matmul(
                        g_ps[kk],
                        lhsT=b_t[:, kk * 128:(kk + 1) * 128],
                        rhs=b_t,
                        start=(t == 0),
                        stop=(t == n_rchunks - 1),
                    )
            for kk in range(KB):
                nc.vector.tensor_copy(g_sb[:, kk, :], g_ps[kk])

            # ---- phase 2: q[r] = a_r^T G a_r, scale = rsqrt(q), bcast -------
            for (n0, nw) in N_CHUNKS:
                q_ps = ps_q.tile([1, NW], f32, tag="q")
                for kk in range(KB):        # output c-half of Tt = G @ A^T
                    tt_ps = ps_acc.tile([128, NW], f32, tag="acc")
                    for k in range(KB):     # contraction over c'
                        nc.tensor.matmul(
                            tt_ps[:, :nw],
                            lhsT=g_sb[:, k, kk * 128:(kk + 1) * 128],
                            rhs=at_sb[:, k, n0:n0 + nw],
                            start=(k == 0),
                            stop=(k == KB - 1),
                        )
                    p_sb = pwork.tile([128, NW], q_dt, tag="p")
                    nc.vector.tensor_mul(p_sb[:, :nw], tt_ps[:, :nw], dve(at_sb[:, kk, n0:n0 + nw]))
                    nc.tensor.matmul(
                        q_ps[:, :nw],
                        lhsT=ones_col,
                        rhs=p_sb[:, :nw],
                        start=(kk == 0),
                        stop=(kk == KB - 1),
                    )
                srow = pwork.tile([1, NW], f32, tag="srow")
                nc.scalar.activation(
                    out=srow[:, :nw], in_=q_ps[:, :nw],
                    func=mybir.ActivationFunctionType.Sqrt,
                )
                # ~51-ULP reciprocal, 5x faster than exact (error << matmul dtype error)
                srow2 = pwork.tile([1, NW], f32, tag="srow2")
                nc.vector.reciprocal_approx_fast(out=srow2[:, :nw], in_=srow[:, :nw])
                srow = srow2
                bc_ps = ps_main.tile([128, NW], f32, tag="main")
                nc.tensor.matmul(
                    bc_ps[:, :nw],
                    lhsT=ones_row,
                    rhs=srow[:, :nw],
                    start=True,
                    stop=True,
                )
                nc.vector.tensor_copy(scale_bc[:, n0:n0 + nw], bc_ps[:, :nw])

            # ---- phase 3: out[shard] = (A_shard @ B^T) * scale[None, :] -----
            # n-outer: chunk n's work starts as soon as its bt chunk + scale
            # chunk are ready, overlapping with the penalty phase.
            for (n0, nw) in N_CHUNKS:
                for (m0, mp) in M_TILES:
                    o_ps = ps_main.tile([128, NW], f32, tag="main")
                    for k in range(KB):
                        nc.tensor.matmul(
                            o_ps[:mp, :nw],
                            lhsT=ash_sb[:, k, m0:m0 + mp],
                            rhs=bt_sb[:, k, n0:n0 + nw],
                            start=(k == 0),
                            stop=(k == KB - 1),
                        )
                    o_sb = outp.tile([128, NW], f32, tag="o")
                    nc.any.tensor_mul(o_sb[:mp, :nw], o_ps[:mp, :nw], scale_bc[:mp, n0:n0 + nw])
                    nc.sync.dma_start(out=out[m0:m0 + mp, n0:n0 + nw], in_=o_sb[:mp, :nw])

    if not nc.is_finalized():
        nc.finalize()
    return nc


_NC_CACHE = {}


def _get_nc(mm_dt_name="float32r", pen_dt_name="bfloat16"):
    key = (mm_dt_name, pen_dt_name)
    if key not in _NC_CACHE:
        _NC_CACHE[key] = build_bass(*key)
    return _NC_CACHE[key]


def _install_ntff_hook_shim():
    """The agent image lacks antenv.axon_hooks; recreate the glue so
    trace=True can capture NTFF profiles via libaxon_pjrt.so ctypes."""
    import types

    name = "antenv.axon_hooks"
    if name in sys.modules:
        return
    hook = None
    try:
        from trn_agent_boot.trn_boot import _ntff_profile_via_ctypes

        hook = _ntff_profile_via_ctypes("/opt/axon/libaxon_pjrt.so")
    except Exception as e:  # pragma: no cover - profiling is best-effort
        print("ntff hook shim unavailable:", e)
    mod = types.ModuleType(name)
    mod._hook = hook
    mod.get_axon_ntff_profile_hook = lambda: mod._hook
    mod.set_axon_ntff_profile_hook = lambda h: setattr(mod, "_hook", h)
    sys.modules[name] = mod
    import antenv

    antenv.axon_hooks = mod


def _run(feature_A, feature_B, trace=False, mm_dt_name="float32r", pen_dt_name="bfloat16"):
    from concourse.bass_utils import run_bass_kernel_spmd

    if trace:
        _install_ntff_hook_shim()

    mm_np = _np_dt(mm_dt_name)
    pen_np = _np_dt(pen_dt_name)

    A = np.ascontiguousarray(np.asarray(feature_A, dtype=np.float32).reshape(R, C))
    B = np.ascontiguousarray(np.asarray(feature_B, dtype=np.float32).reshape(R, C))
    AT = np.ascontiguousarray(A.T)
    BT = np.ascontiguousarray(B.T).astype(mm_np)

    at_in = AT.astype(pen_np)
    b_in = B.astype(pen_np)

    in_maps = []
    for d in range(N_CORES):
        in_maps.append({
            "at": at_in,
            "bt": BT,
            "b": b_in,
            "ashard": np.ascontiguousarray(AT[:, d * S:(d + 1) * S]).astype(mm_np),
        })

    nc = _get_nc(mm_dt_name, pen_dt_name)
    res = run_bass_kernel_spmd(nc, in_maps, core_ids=list(range(N_CORES)), trace=trace)
    full = np.concatenate([r["out"] for r in res.results], axis=0)  # [R, R]
    return full.reshape(1, H, W, H, W), res


def kernel(feature_A, feature_B):
    out, _ = _run(feature_A, feature_B, trace=False)
    return out
